# revision 13
# baseline (speedup 1.0000x reference)
"""Trainium2 Bass kernel for nn_CrossAttention (B=4, H=8, D=64, C=512, N=M=2048).

Sharding: 8 cores = batch (4) x head-group (2). Core c handles batch b=c//2
and heads hg*4..hg*4+4 with hg=c%2 (tensor parallel on inner_dim). Each core
emits a full-shape partial y (its Wo column block times its heads' attention
output, plus bias on hg==0); the host unshard sums the two partials per batch.

V2 design notes (evolved from the fp32r baseline, 211us -> target ~125us):
  * Everything on the PE runs bf16 (fp32 "HIGH" mode streams ~1.33 cyc/col;
    bf16 streams 1.0 and enables FWL weight loads).  Exp output p is bf16, so
    the pv matmuls are bf16 too.
  * pv keeps the ones-augmented lhsT (row 64 = softmax denominator riding the
    matmul for free) and accumulates fp32 in PSUM across the 16 j-chunks.
  * y projection accumulates both inner chunks in PSUM (start/stop pair), no
    SBUF y accumulator.
  * Optionally (SCHRAUD_JS) some exp chunks run on the DVE as a Schraudolph
    bit-trick (i16 = round(A*s + B) viewed as bf16 ~= e^s), offloading the
    scalar engine, which is otherwise the ~1100ns/j bottleneck vs the PE's
    ~840ns/j.

Per-core math:
  q  = Wq[hg] @ x             [256, 2048]
  k  = Wk[hg] @ ctx           [256, 2048]
  vT = ctx.T @ Wv[hg].T       [2048, 256]  (stored j-chunked, ones-augmented)
  per local head h: simT[j,i] = sum_d k[d,j] q[d,i];  p = exp(simT/8)
  out_aug = [vT_h | 1].T @ p  [65, 2048]   (row 64 = softmax denominator)
  out_h   = out_aug[:64] / out_aug[64]
  y_part = Wo[:, hg cols] @ out (+ bo)     [512, 2048]
"""

from collections import deque
from contextlib import ExitStack

import numpy as np

import concourse.bass as bass
import concourse.mybir as mybir
import concourse.tile as tile
from concourse import bacc
from concourse.bass_utils import run_bass_kernel_spmd

FP = mybir.dt.float32
BF16 = mybir.dt.bfloat16
I16 = mybir.dt.int16
EXP = mybir.ActivationFunctionType.Exp

P = 128
H, D = 8, 64
C = 512             # query_dim == full inner_dim
N, M = 2048, 2048
HL = 4              # local heads per core
HPL = 2             # local head pairs
CIN = HL * D        # local inner dim = 256
CC = C // P         # 4 contraction chunks for q/k/v projections
IT = N // 512       # 4 query i-tiles
JC = M // P         # 16 context chunks
GC = JC // 2        # 8 j-chunk pairs per pass (one exp instr each)
NT = M // 512       # 4 context column blocks
ICY = CIN // P      # 2 inner chunks for the y projection
SCALE = float(D) ** -0.5
N_CORES = 8
NWARM = 3

# j-chunk indices (0..15) whose exp runs on the DVE via the Schraudolph
# bit-trick instead of the scalar engine.  () = all-ACT (exact exp).
SCHRAUD_JS = ()
# i16 = SCH_A * s + SCH_B, bitcast to bf16 ~= exp(SCALE * s).
SCH_A = SCALE * (2.0 ** 7) / float(np.log(2.0))
SCH_B = 127.0 * (2.0 ** 7) - 5.72


def _build_program():
    nc = bacc.Bacc("TRN2")
    x = nc.dram_tensor("x", [P, IT * CC * 512], BF16, kind="ExternalInput")
    ctx = nc.dram_tensor("ctx", [P, NT * CC * 512], BF16, kind="ExternalInput")
    wq = nc.dram_tensor("wq", [P, CC * CIN], BF16, kind="ExternalInput")
    wk = nc.dram_tensor("wk", [P, CC * CIN], BF16, kind="ExternalInput")
    wv = nc.dram_tensor("wv", [P, CC * CIN], BF16, kind="ExternalInput")
    wo = nc.dram_tensor("wo", [P, ICY * 512], BF16, kind="ExternalInput")
    bo = nc.dram_tensor("bo", [P, CC], FP, kind="ExternalInput")
    y = nc.dram_tensor("y", [P, CC * N], FP, kind="ExternalOutput")

    with tile.TileContext(nc) as tc:
        _emit(tc, x, ctx, wq, wk, wv, wo, bo, y)
    nc.finalize()
    return nc


def _emit(tc, x, ctx, wq, wk, wv, wo, bo, y):
    nc = tc.nc
    with ExitStack() as st:
        wpool = st.enter_context(tc.tile_pool(name="weights", bufs=1))
        apool = st.enter_context(tc.tile_pool(name="acts", bufs=1))
        ppool = st.enter_context(tc.tile_pool(name="pexp", bufs=3))
        spool = st.enter_context(tc.tile_pool(name="small", bufs=2))
        ypool = st.enter_context(tc.tile_pool(name="ystage", bufs=4))
        psim = st.enter_context(tc.tile_pool(name="psim", bufs=2, space="PSUM"))
        ppv = st.enter_context(tc.tile_pool(name="ppv", bufs=2, space="PSUM"))
        pmisc = st.enter_context(tc.tile_pool(name="pmisc", bufs=2, space="PSUM"))

        # ---- input loads, ordered so pass(0,0) starts ASAP ----
        wq_s = wpool.tile([P, CC * CIN], BF16, tag="wq")
        nc.sync.dma_start(out=wq_s, in_=wq[:, :])
        x_s = apool.tile([P, IT * CC * 512], BF16, tag="x")
        nc.sync.dma_start(out=x_s[:, 0:2048], in_=x[:, 0:2048])
        wk_s = wpool.tile([P, CC * CIN], BF16, tag="wk")
        nc.sync.dma_start(out=wk_s, in_=wk[:, :])
        ctx_s = apool.tile([P, NT * CC * 512], BF16, tag="ctx")
        nc.sync.dma_start(out=ctx_s[:, 0:2048], in_=ctx[:, 0:2048])
        wv_s = wpool.tile([P, CC * CIN], BF16, tag="wv")
        nc.sync.dma_start(out=wv_s, in_=wv[:, :])
        for nb in range(1, NT):
            nc.sync.dma_start(
                out=ctx_s[:, nb * 2048:(nb + 1) * 2048],
                in_=ctx[:, nb * 2048:(nb + 1) * 2048],
            )
        for it in range(1, IT):
            nc.sync.dma_start(
                out=x_s[:, it * 2048:(it + 1) * 2048],
                in_=x[:, it * 2048:(it + 1) * 2048],
            )
        wo_s = wpool.tile([P, ICY * 512], BF16, tag="wo")
        nc.sync.dma_start(out=wo_s, in_=wo[:, :])
        bo_s = wpool.tile([P, CC], FP, tag="bo")
        nc.sync.dma_start(out=bo_s, in_=bo[:, :])

        # ---- persistent SBUF intermediates ----
        # q/k: local head pair hp at cols hp*2048 + it(or nt)*512 + n
        q_s = apool.tile([P, HPL * N], BF16, tag="q")
        k_s = apool.tile([P, HPL * M], BF16, tag="k")
        # v aug: j-chunk j at cols j*(HL*65), local head h at sub-cols h*65
        vaug = apool.tile([P, JC * (HL * 65)], BF16, tag="vaug")
        # attention out, local inner chunk ic at cols ic*2048 + it*512
        out_s = apool.tile([P, ICY * N], BF16, tag="out")
        ones_s = wpool.tile([P, P], BF16, tag="ones")
        nc.vector.memset(ones_s, 1.0)
        ones_f = wpool.tile([P, P], FP, tag="onesf")
        nc.vector.memset(ones_f, 1.0)
        vaug4 = vaug.rearrange("p (j h e) -> p j h e", j=JC, h=HL)
        ones4 = ones_s[:, 0:JC * HL].rearrange("p (j h e) -> p j h e", j=JC, h=HL)
        nc.vector.tensor_copy(out=vaug4[:, :, :, 64:65], in_=ones4)

        # HAM warmup: burn matmuls on the ones tile during the initial DMA
        # wait so the first projections run at the full 2.4GHz clock.
        warm = pmisc.tile([P, 512], FP, tag="scratch", name="warm")
        for w in range(NWARM):
            nc.tensor.matmul(warm[:, 0:P], lhsT=ones_s[:, 0:P],
                             rhs=ones_s[:, 0:P],
                             start=(w == 0), stop=(w == NWARM - 1))
        warm_sink = spool.tile([P, P], FP, tag="warmsink", bufs=1)
        nc.vector.tensor_copy(out=warm_sink, in_=warm[:, 0:P])

        def proj_qk(dst, w_s, oc, rhs_of_cc):
            """One [128, 512] q/k projection tile (local head pair oc)."""
            pt = pmisc.tile([P, 512], FP, tag="scratch")
            for cc in range(CC):
                nc.tensor.matmul(
                    pt,
                    lhsT=w_s[:, cc * CIN + oc * P: cc * CIN + (oc + 1) * P],
                    rhs=rhs_of_cc(cc),
                    start=(cc == 0), stop=(cc == CC - 1),
                )
            nc.vector.tensor_copy(out=dst, in_=pt)

        def emit_q(oc, it):
            proj_qk(q_s[:, oc * N + it * 512: oc * N + (it + 1) * 512], wq_s, oc,
                    lambda cc: x_s[:, it * 2048 + cc * 512: it * 2048 + (cc + 1) * 512])

        def emit_k(oc, nt):
            proj_qk(k_s[:, oc * M + nt * 512: oc * M + (nt + 1) * 512], wk_s, oc,
                    lambda cc: ctx_s[:, nt * 2048 + cc * 512: nt * 2048 + (cc + 1) * 512])

        def emit_v(j):
            nb, jm = j // 4, j % 4
            pt = pmisc.tile([P, 512], FP, tag="scratch")
            for cc in range(CC):
                nc.tensor.matmul(
                    pt[:, 0:CIN],
                    lhsT=ctx_s[:, nb * 2048 + cc * 512 + jm * P:
                               nb * 2048 + cc * 512 + (jm + 1) * P],
                    rhs=wv_s[:, cc * CIN:(cc + 1) * CIN],
                    start=(cc == 0), stop=(cc == CC - 1),
                )
            nc.vector.tensor_copy(
                out=vaug4[:, j, :, 0:64],
                in_=pt[:, 0:CIN].rearrange("p (h e) -> p h e", h=HL),
            )

        def emit_y(oc, nt2):
            """Full y output chunk: both inner chunks accumulated in PSUM."""
            pt = pmisc.tile([P, 512], FP, tag="scratch")
            for ic in range(ICY):
                nc.tensor.matmul(
                    pt,
                    lhsT=wo_s[:, ic * 512 + oc * P: ic * 512 + (oc + 1) * P],
                    rhs=out_s[:, ic * N + nt2 * 512: ic * N + (nt2 + 1) * 512],
                    start=(ic == 0), stop=(ic == ICY - 1),
                )
            ys = ypool.tile([P, 512], FP, tag="ys")
            nc.vector.tensor_scalar_add(out=ys, in0=pt, scalar1=bo_s[:, oc:oc + 1])
            nc.sync.dma_start(
                out=y[:, oc * N + nt2 * 512: oc * N + (nt2 + 1) * 512], in_=ys)

        # pinned[i]: projection tiles that MUST be emitted during pass i
        # (they feed later js of pass i or pass i+1); free: y chunks drained
        # opportunistically.
        pinned = {i: deque() for i in range(HPL * IT)}
        pinned[0].append(lambda: emit_k(0, 1))
        pinned[0].append(lambda: emit_k(0, 2))
        pinned[0].append(lambda: emit_k(0, 3))
        pinned[0].append(lambda: emit_q(0, 1))
        pinned[1].append(lambda: emit_q(0, 2))
        pinned[1].append(lambda: emit_k(1, 0))
        pinned[1].append(lambda: emit_k(1, 1))
        pinned[2].append(lambda: emit_q(0, 3))
        pinned[2].append(lambda: emit_k(1, 2))
        pinned[2].append(lambda: emit_k(1, 3))
        pinned[2].append(lambda: emit_q(1, 0))
        for it in range(1, IT):
            pinned[2 + it].append(lambda it=it: emit_q(1, it))
        free = deque()

        # upfront: only what pass (0,0)'s first js need
        emit_q(0, 0)
        emit_k(0, 0)

        def pass_begin(hp, it, emit_v_inline):
            """Allocate pass state and emit sim+exp for j=0 (hoisted above the
            previous pass's pv(15)+norm so the ACT exp stream never stalls at
            a pass boundary)."""
            c = {}
            c["hp"], c["it"], c["v_inline"] = hp, it, emit_v_inline
            c["pvA"] = ppv.tile([65, 512], FP, tag="pv", name="pvA")
            c["pvB"] = ppv.tile([65, 512], FP, tag="pv", name="pvB")
            c["qA"] = q_s[0:64, hp * N + it * 512: hp * N + (it + 1) * 512]
            c["qB"] = q_s[64:128, hp * N + it * 512: hp * N + (it + 1) * 512]
            c["jtiles"] = [None] * JC
            emit_sim(c, 0)
            emit_exp(c, 0)
            return c

        def emit_sim(c, j):
            # sim for j-chunk: 2 packed matmuls (heads A/B on row groups
            # 0-1 / 2-3) into one [128, 1024] fp32 psum tile (2 banks).
            hp = c["hp"]
            stt = psim.tile([P, 1024], FP, tag="sim", name="st_t")
            pt = ppool.tile([P, 1024], BF16, tag="p", name="pt")
            c["jtiles"][j] = (stt, pt)
            for half in range(2):
                nc.tensor.matmul(
                    stt[:, half * 512:(half + 1) * 512],
                    lhsT=k_s[half * 64:(half + 1) * 64,
                             hp * M + j * P: hp * M + (j + 1) * P],
                    rhs=(c["qA"] if half == 0 else c["qB"]),
                )
            if c["v_inline"]:
                emit_v(j)

        def emit_exp(c, j):
            stt, pt = c["jtiles"][j]
            if j in SCHRAUD_JS:
                nc.vector.tensor_scalar(
                    out=pt.bitcast(I16), in0=stt,
                    scalar1=SCH_A, scalar2=SCH_B,
                    op0=mybir.AluOpType.mult, op1=mybir.AluOpType.add)
            else:
                nc.scalar.activation(out=pt, in_=stt, func=EXP, scale=SCALE)

        def emit_pv(c, j):
            pt = c["jtiles"][j][1]
            hA, hB = 2 * c["hp"], 2 * c["hp"] + 1
            for half in range(2):
                h = hA if half == 0 else hB
                nc.tensor.matmul(
                    c["pvA"] if half == 0 else c["pvB"],
                    lhsT=vaug[:, j * (HL * 65) + h * 65:
                              j * (HL * 65) + h * 65 + 65],
                    rhs=pt[:, half * 512:(half + 1) * 512],
                    start=(j == 0), stop=(j == JC - 1),
                )

        def pass_main(c, mine):
            for j in range(JC - 1):
                if j > 0:  # (sim(1), exp(1)) follow the previous pass finish
                    emit_sim(c, j + 1)
                    emit_exp(c, j + 1)
                emit_pv(c, j)
                if j % 2 == 1:
                    if mine:
                        mine.popleft()()
                    elif free:
                        free.popleft()()

        def pass_finish(c):
            """pv(15) + pv drains + normalization (emitted after the NEXT
            pass's first sim+exp).  The softmax denominators sit at psum
            partition 64 of pvA/pvB; reciprocal runs in-lane, then a K=1
            matmul with a ones lhsT broadcasts 1/den across all partitions
            (cheaper and lower-latency than gpsimd partition_broadcast)."""
            emit_pv(c, JC - 1)
            pvA, pvB, hp, it = c["pvA"], c["pvB"], c["hp"], c["it"]
            raw = spool.tile([P, 1024], FP, tag="raw", bufs=2)
            nc.vector.tensor_copy(out=raw[0:65, 0:512], in_=pvA)
            nc.vector.tensor_copy(out=raw[0:65, 512:1024], in_=pvB)
            den = spool.tile([1, 1024], FP, tag="den", bufs=2)
            nc.sync.dma_start(out=den, in_=raw[64:65, 0:1024])
            nc.vector.reciprocal_approx_fast(out=den[0:1, 0:512],
                                             in_=den[0:1, 0:512])
            nc.vector.reciprocal_approx_fast(out=den[0:1, 512:1024],
                                             in_=den[0:1, 512:1024])
            bcA = pmisc.tile([P, 512], FP, tag="scratch")
            nc.tensor.matmul(bcA, lhsT=ones_f[0:1, 0:P], rhs=den[0:1, 0:512])
            bcB = pmisc.tile([P, 512], FP, tag="scratch")
            nc.tensor.matmul(bcB, lhsT=ones_f[0:1, 0:P], rhs=den[0:1, 512:1024])
            bb = spool.tile([P, 512], FP, tag="bshift", bufs=2)
            nc.sync.dma_start(out=bb[64:128, :], in_=raw[0:64, 512:1024])
            ocol = hp * N + it * 512
            nc.vector.tensor_mul(out=out_s[0:64, ocol:ocol + 512],
                                 in0=raw[0:64, 0:512], in1=bcA[0:64, :])
            nc.vector.tensor_mul(out=out_s[64:128, ocol:ocol + 512],
                                 in0=bb[64:128, :], in1=bcB[64:128, :])
            if hp == 1:
                for oc in range(CC):
                    free.append(lambda oc=oc, it=it: emit_y(oc, it))

        prev = None
        for hp in range(HPL):
            for it in range(IT):
                c = pass_begin(hp, it, emit_v_inline=(hp == 0 and it == 0))
                if prev is not None:
                    pass_finish(prev)
                emit_sim(c, 1)
                emit_exp(c, 1)
                pass_main(c, pinned[hp * IT + it])
                prev = c
        pass_finish(prev)
        while free:
            free.popleft()()


# ------------------------- host-side shard / gather -------------------------

def _bf16(a):
    import ml_dtypes
    return np.ascontiguousarray(a.astype(ml_dtypes.bfloat16))


def _shard_inputs(x, context, Wq, Wk, Wv, Wo, bo):
    """Build the per-core DRAM images (all [128, free])."""
    def chunk_rows(a):
        n = a.shape[1]
        return np.ascontiguousarray(
            a.reshape(-1, P, n).transpose(1, 0, 2).reshape(P, -1))

    WqT, WkT, WvT, WoT = Wq.T, Wk.T, Wv.T, Wo.T
    zeros_bo = np.zeros((P, CC), np.float32)

    in_maps = []
    for c in range(N_CORES):
        b, hg = c // 2, c % 2
        cols = slice(hg * CIN, (hg + 1) * CIN)
        x_s = x[b].reshape(CC, P, IT, 512).transpose(1, 2, 0, 3).reshape(P, IT * CC * 512)
        ctx_s = context[b].reshape(CC, P, NT, 512).transpose(1, 2, 0, 3).reshape(P, NT * CC * 512)
        in_maps.append({
            "x": _bf16(x_s),
            "ctx": _bf16(ctx_s),
            "wq": _bf16(chunk_rows(np.ascontiguousarray(WqT[:, cols]))),
            "wk": _bf16(chunk_rows(np.ascontiguousarray(WkT[:, cols]))),
            "wv": _bf16(chunk_rows(np.ascontiguousarray(WvT[:, cols]))),
            "wo": _bf16(chunk_rows(np.ascontiguousarray(WoT[hg * CIN:(hg + 1) * CIN, :]))),
            "bo": np.ascontiguousarray(bo.reshape(CC, P).T) if hg == 0 else zeros_bo,
        })
    return in_maps


def _gather_outputs(results):
    y_full = np.empty((4, C, N), np.float32)
    for b in range(4):
        acc = None
        for hg in range(2):
            y_s = results[2 * b + hg]["y"]                    # [128, 4*2048]
            part = y_s.reshape(P, CC, N).transpose(1, 0, 2).reshape(C, N)
            acc = part if acc is None else acc + part
        y_full[b] = acc
    return y_full


_PROGRAM = None


def _get_program():
    global _PROGRAM
    if _PROGRAM is None:
        _PROGRAM = _build_program()
    return _PROGRAM


def run(trace=False, **inputs):
    nc = _get_program()
    in_maps = _shard_inputs(
        np.asarray(inputs["x"], np.float32),
        np.asarray(inputs["context"], np.float32),
        np.asarray(inputs["Wq"], np.float32),
        np.asarray(inputs["Wk"], np.float32),
        np.asarray(inputs["Wv"], np.float32),
        np.asarray(inputs["Wo"], np.float32),
        np.asarray(inputs["bo"], np.float32),
    )
    res = run_bass_kernel_spmd(nc, in_maps, list(range(N_CORES)), trace=trace)
    return _gather_outputs(res.results), res


def kernel(**inputs):
    out, _ = run(trace=False, **inputs)
    return out


# revision 16
# speedup vs baseline: 1.1524x; 1.1524x over previous
"""Trainium2 Bass kernel for nn_CrossAttention (B=4, H=8, D=64, C=512, N=M=2048).

Sharding: 8 cores = batch (4) x head-group (2). Core c handles batch b=c//2
and heads hg*4..hg*4+4 with hg=c%2 (tensor parallel on inner_dim). Each core
emits a full-shape partial y (its Wo column block times its heads' attention
output, plus bias on hg==0); the host unshard sums the two partials per batch.

V2 design notes (evolved from the fp32r baseline, 211us -> target ~125us):
  * Everything on the PE runs bf16 (fp32 "HIGH" mode streams ~1.33 cyc/col;
    bf16 streams 1.0 and enables FWL weight loads).  Exp output p is bf16, so
    the pv matmuls are bf16 too.
  * pv keeps the ones-augmented lhsT (row 64 = softmax denominator riding the
    matmul for free) and accumulates fp32 in PSUM across the 16 j-chunks.
  * y projection accumulates both inner chunks in PSUM (start/stop pair), no
    SBUF y accumulator.
  * Optionally (SCHRAUD_JS) some exp chunks run on the DVE as a Schraudolph
    bit-trick (i16 = round(A*s + B) viewed as bf16 ~= e^s), offloading the
    scalar engine, which is otherwise the ~1100ns/j bottleneck vs the PE's
    ~840ns/j.

Per-core math:
  q  = Wq[hg] @ x             [256, 2048]
  k  = Wk[hg] @ ctx           [256, 2048]
  vT = ctx.T @ Wv[hg].T       [2048, 256]  (stored j-chunked, ones-augmented)
  per local head h: simT[j,i] = sum_d k[d,j] q[d,i];  p = exp(simT/8)
  out_aug = [vT_h | 1].T @ p  [65, 2048]   (row 64 = softmax denominator)
  out_h   = out_aug[:64] / out_aug[64]
  y_part = Wo[:, hg cols] @ out (+ bo)     [512, 2048]
"""

from collections import deque
from contextlib import ExitStack

import numpy as np

import concourse.bass as bass
import concourse.mybir as mybir
import concourse.tile as tile
from concourse import bacc
from concourse.bass_utils import run_bass_kernel_spmd

FP = mybir.dt.float32
BF16 = mybir.dt.bfloat16
I16 = mybir.dt.int16
EXP = mybir.ActivationFunctionType.Exp

P = 128
H, D = 8, 64
C = 512             # query_dim == full inner_dim
N, M = 2048, 2048
HL = 4              # local heads per core
HPL = 2             # local head pairs
CIN = HL * D        # local inner dim = 256
CC = C // P         # 4 contraction chunks for q/k/v projections
IT = N // 512       # 4 query i-tiles
JC = M // P         # 16 context chunks
GC = JC // 2        # 8 j-chunk pairs per pass (one exp instr each)
NT = M // 512       # 4 context column blocks
ICY = CIN // P      # 2 inner chunks for the y projection
SCALE = float(D) ** -0.5
N_CORES = 8
NWARM = 6

# j-chunk indices (0..15) whose exp runs on the DVE via the Schraudolph
# bit-trick instead of the scalar engine.  () = all-ACT (exact exp).
SCHRAUD_JS = ()
# i16 = SCH_A * s + SCH_B, bitcast to bf16 ~= exp(SCALE * s).
SCH_A = SCALE * (2.0 ** 7) / float(np.log(2.0))
SCH_B = 127.0 * (2.0 ** 7) - 5.72


def _build_program():
    nc = bacc.Bacc("TRN2")
    x = nc.dram_tensor("x", [P, IT * CC * 512], BF16, kind="ExternalInput")
    ctx = nc.dram_tensor("ctx", [P, NT * CC * 512], BF16, kind="ExternalInput")
    wq = nc.dram_tensor("wq", [P, CC * CIN], BF16, kind="ExternalInput")
    wk = nc.dram_tensor("wk", [P, CC * CIN], BF16, kind="ExternalInput")
    wv = nc.dram_tensor("wv", [P, CC * CIN], BF16, kind="ExternalInput")
    wo = nc.dram_tensor("wo", [P, ICY * 512], BF16, kind="ExternalInput")
    bo = nc.dram_tensor("bo", [P, CC], FP, kind="ExternalInput")
    y = nc.dram_tensor("y", [P, CC * N], FP, kind="ExternalOutput")

    with tile.TileContext(nc) as tc:
        _emit(tc, x, ctx, wq, wk, wv, wo, bo, y)
    nc.finalize()
    return nc


def _emit(tc, x, ctx, wq, wk, wv, wo, bo, y):
    nc = tc.nc
    with ExitStack() as st:
        wpool = st.enter_context(tc.tile_pool(name="weights", bufs=1))
        apool = st.enter_context(tc.tile_pool(name="acts", bufs=1))
        ppool = st.enter_context(tc.tile_pool(name="pexp", bufs=3))
        spool = st.enter_context(tc.tile_pool(name="small", bufs=2))
        ypool = st.enter_context(tc.tile_pool(name="ystage", bufs=4))
        psim = st.enter_context(tc.tile_pool(name="psim", bufs=2, space="PSUM"))
        ppv = st.enter_context(tc.tile_pool(name="ppv", bufs=2, space="PSUM"))
        pmisc = st.enter_context(tc.tile_pool(name="pmisc", bufs=2, space="PSUM"))

        # ---- input loads, split across both HWDGE queues (SP carries the
        # q path, ACT carries the k/v path) so pass(0,0) starts ASAP ----
        wq_s = wpool.tile([P, CC * CIN], BF16, tag="wq")
        nc.sync.dma_start(out=wq_s, in_=wq[:, :])
        wk_s = wpool.tile([P, CC * CIN], BF16, tag="wk")
        nc.scalar.dma_start(out=wk_s, in_=wk[:, :])
        x_s = apool.tile([P, IT * CC * 512], BF16, tag="x")
        nc.sync.dma_start(out=x_s[:, 0:2048], in_=x[:, 0:2048])
        ctx_s = apool.tile([P, NT * CC * 512], BF16, tag="ctx")
        nc.scalar.dma_start(out=ctx_s[:, 0:2048], in_=ctx[:, 0:2048])
        wv_s = wpool.tile([P, CC * CIN], BF16, tag="wv")
        nc.scalar.dma_start(out=wv_s, in_=wv[:, :])
        for nb in range(1, NT):
            nc.scalar.dma_start(
                out=ctx_s[:, nb * 2048:(nb + 1) * 2048],
                in_=ctx[:, nb * 2048:(nb + 1) * 2048],
            )
        for it in range(1, IT):
            nc.sync.dma_start(
                out=x_s[:, it * 2048:(it + 1) * 2048],
                in_=x[:, it * 2048:(it + 1) * 2048],
            )
        wo_s = wpool.tile([P, ICY * 512], BF16, tag="wo")
        nc.sync.dma_start(out=wo_s, in_=wo[:, :])
        bo_s = wpool.tile([P, CC], FP, tag="bo")
        nc.sync.dma_start(out=bo_s, in_=bo[:, :])

        # ---- persistent SBUF intermediates ----
        # q/k: local head pair hp at cols hp*2048 + it(or nt)*512 + n
        q_s = apool.tile([P, HPL * N], BF16, tag="q")
        k_s = apool.tile([P, HPL * M], BF16, tag="k")
        # v aug: j-chunk j at cols j*(HL*65), local head h at sub-cols h*65
        vaug = apool.tile([P, JC * (HL * 65)], BF16, tag="vaug")
        # attention out, local inner chunk ic at cols ic*2048 + it*512
        out_s = apool.tile([P, ICY * N], BF16, tag="out")
        ones_s = wpool.tile([P, P], BF16, tag="ones")
        nc.vector.memset(ones_s, 1.0)
        ones_f = wpool.tile([P, P], FP, tag="onesf")
        nc.vector.memset(ones_f, 1.0)
        vaug4 = vaug.rearrange("p (j h e) -> p j h e", j=JC, h=HL)
        ones4 = ones_s[:, 0:JC * HL].rearrange("p (j h e) -> p j h e", j=JC, h=HL)
        nc.vector.tensor_copy(out=vaug4[:, :, :, 64:65], in_=ones4)

        # HAM warmup: burn matmuls on the ones tile during the initial DMA
        # wait so the first projections run at the full 2.4GHz clock.
        warm = pmisc.tile([P, 512], FP, tag="scratch", name="warm")
        for w in range(NWARM):
            nc.tensor.matmul(warm[:, 0:P], lhsT=ones_s[:, 0:P],
                             rhs=ones_s[:, 0:P],
                             start=(w == 0), stop=(w == NWARM - 1))
        warm_sink = spool.tile([P, P], FP, tag="warmsink", bufs=1)
        nc.vector.tensor_copy(out=warm_sink, in_=warm[:, 0:P])

        def proj_qk(dst, w_s, oc, rhs_of_cc):
            """One [128, 512] q/k projection tile (local head pair oc)."""
            pt = pmisc.tile([P, 512], FP, tag="scratch")
            for cc in range(CC):
                nc.tensor.matmul(
                    pt,
                    lhsT=w_s[:, cc * CIN + oc * P: cc * CIN + (oc + 1) * P],
                    rhs=rhs_of_cc(cc),
                    start=(cc == 0), stop=(cc == CC - 1),
                )
            nc.vector.tensor_copy(out=dst, in_=pt)

        def emit_q(oc, it):
            proj_qk(q_s[:, oc * N + it * 512: oc * N + (it + 1) * 512], wq_s, oc,
                    lambda cc: x_s[:, it * 2048 + cc * 512: it * 2048 + (cc + 1) * 512])

        def emit_k(oc, nt):
            proj_qk(k_s[:, oc * M + nt * 512: oc * M + (nt + 1) * 512], wk_s, oc,
                    lambda cc: ctx_s[:, nt * 2048 + cc * 512: nt * 2048 + (cc + 1) * 512])

        def emit_v(j):
            nb, jm = j // 4, j % 4
            pt = pmisc.tile([P, 512], FP, tag="scratch")
            for cc in range(CC):
                nc.tensor.matmul(
                    pt[:, 0:CIN],
                    lhsT=ctx_s[:, nb * 2048 + cc * 512 + jm * P:
                               nb * 2048 + cc * 512 + (jm + 1) * P],
                    rhs=wv_s[:, cc * CIN:(cc + 1) * CIN],
                    start=(cc == 0), stop=(cc == CC - 1),
                )
            nc.vector.tensor_copy(
                out=vaug4[:, j, :, 0:64],
                in_=pt[:, 0:CIN].rearrange("p (h e) -> p h e", h=HL),
            )

        def emit_y(oc, nt2):
            """Full y output chunk: both inner chunks accumulated in PSUM."""
            pt = pmisc.tile([P, 512], FP, tag="scratch")
            for ic in range(ICY):
                nc.tensor.matmul(
                    pt,
                    lhsT=wo_s[:, ic * 512 + oc * P: ic * 512 + (oc + 1) * P],
                    rhs=out_s[:, ic * N + nt2 * 512: ic * N + (nt2 + 1) * 512],
                    start=(ic == 0), stop=(ic == ICY - 1),
                )
            ys = ypool.tile([P, 512], FP, tag="ys")
            nc.vector.tensor_scalar_add(out=ys, in0=pt, scalar1=bo_s[:, oc:oc + 1])
            nc.sync.dma_start(
                out=y[:, oc * N + nt2 * 512: oc * N + (nt2 + 1) * 512], in_=ys)

        # pinned[i]: projection tiles that MUST be emitted during pass i
        # (they feed later js of pass i or pass i+1); free: y chunks drained
        # opportunistically.
        pinned = {i: deque() for i in range(HPL * IT)}
        pinned[0].append(lambda: emit_k(0, 1))
        pinned[0].append(lambda: emit_k(0, 2))
        pinned[0].append(lambda: emit_k(0, 3))
        pinned[0].append(lambda: emit_q(0, 1))
        pinned[1].append(lambda: emit_q(0, 2))
        pinned[1].append(lambda: emit_k(1, 0))
        pinned[1].append(lambda: emit_k(1, 1))
        pinned[2].append(lambda: emit_q(0, 3))
        pinned[2].append(lambda: emit_k(1, 2))
        pinned[2].append(lambda: emit_k(1, 3))
        pinned[2].append(lambda: emit_q(1, 0))
        for it in range(1, IT):
            pinned[2 + it].append(lambda it=it: emit_q(1, it))
        free = deque()

        # upfront: only what pass (0,0)'s first js need
        emit_q(0, 0)
        emit_k(0, 0)

        def pass_begin(hp, it, emit_v_inline):
            """Allocate pass state and emit sim+exp for j=0 (hoisted above the
            previous pass's pv(15)+norm so the ACT exp stream never stalls at
            a pass boundary)."""
            c = {}
            c["hp"], c["it"], c["v_inline"] = hp, it, emit_v_inline
            c["pvA"] = ppv.tile([65, 512], FP, tag="pv", name="pvA")
            c["pvB"] = ppv.tile([65, 512], FP, tag="pv", name="pvB")
            c["qA"] = q_s[0:64, hp * N + it * 512: hp * N + (it + 1) * 512]
            c["qB"] = q_s[64:128, hp * N + it * 512: hp * N + (it + 1) * 512]
            c["jtiles"] = [None] * JC
            emit_sim(c, 0)
            emit_exp(c, 0)
            return c

        def emit_sim(c, j):
            # sim for j-chunk: 2 packed matmuls (heads A/B on row groups
            # 0-1 / 2-3) into one [128, 1024] fp32 psum tile (2 banks).
            hp = c["hp"]
            stt = psim.tile([P, 1024], FP, tag="sim", name="st_t")
            pt = ppool.tile([P, 1024], BF16, tag="p", name="pt")
            c["jtiles"][j] = (stt, pt)
            for half in range(2):
                nc.tensor.matmul(
                    stt[:, half * 512:(half + 1) * 512],
                    lhsT=k_s[half * 64:(half + 1) * 64,
                             hp * M + j * P: hp * M + (j + 1) * P],
                    rhs=(c["qA"] if half == 0 else c["qB"]),
                )
            if c["v_inline"]:
                emit_v(j)

        def emit_exp(c, j):
            stt, pt = c["jtiles"][j]
            if j in SCHRAUD_JS:
                nc.vector.tensor_scalar(
                    out=pt.bitcast(I16), in0=stt,
                    scalar1=SCH_A, scalar2=SCH_B,
                    op0=mybir.AluOpType.mult, op1=mybir.AluOpType.add)
            else:
                nc.scalar.activation(out=pt, in_=stt, func=EXP, scale=SCALE)

        def emit_pv(c, j):
            pt = c["jtiles"][j][1]
            hA, hB = 2 * c["hp"], 2 * c["hp"] + 1
            for half in range(2):
                h = hA if half == 0 else hB
                nc.tensor.matmul(
                    c["pvA"] if half == 0 else c["pvB"],
                    lhsT=vaug[:, j * (HL * 65) + h * 65:
                              j * (HL * 65) + h * 65 + 65],
                    rhs=pt[:, half * 512:(half + 1) * 512],
                    start=(j == 0), stop=(j == JC - 1),
                )

        def pass_main(c, mine):
            for j in range(JC - 1):
                if j > 0:  # (sim(1), exp(1)) follow the previous pass finish
                    emit_sim(c, j + 1)
                    emit_exp(c, j + 1)
                emit_pv(c, j)
                if j % 2 == 1:
                    if mine:
                        mine.popleft()()
                    elif free:
                        free.popleft()()

        def pass_finish(c):
            """pv(15) + pv drains + normalization (emitted after the NEXT
            pass's first sim+exp).  The softmax denominators sit at psum
            partition 64 of pvA/pvB; reciprocal runs in-lane, then a K=1
            matmul with a ones lhsT broadcasts 1/den across all partitions
            (cheaper and lower-latency than gpsimd partition_broadcast)."""
            emit_pv(c, JC - 1)
            pvA, pvB, hp, it = c["pvA"], c["pvB"], c["hp"], c["it"]
            raw = spool.tile([P, 1024], FP, tag="raw", bufs=2)
            nc.vector.tensor_copy(out=raw[0:65, 0:512], in_=pvA)
            nc.vector.tensor_copy(out=raw[0:65, 512:1024], in_=pvB)
            den = spool.tile([1, 1024], FP, tag="den", bufs=2)
            nc.sync.dma_start(out=den, in_=raw[64:65, 0:1024])
            nc.vector.reciprocal_approx_fast(out=den[0:1, 0:512],
                                             in_=den[0:1, 0:512])
            nc.vector.reciprocal_approx_fast(out=den[0:1, 512:1024],
                                             in_=den[0:1, 512:1024])
            bcA = spool.tile([P, 512], FP, tag="bc", bufs=2)
            bcB = spool.tile([P, 512], FP, tag="bc", bufs=2)
            nc.gpsimd.partition_broadcast(bcA, den[0:1, 0:512])
            nc.gpsimd.partition_broadcast(bcB, den[0:1, 512:1024])
            bb = spool.tile([P, 512], FP, tag="bshift", bufs=2)
            nc.sync.dma_start(out=bb[64:128, :], in_=raw[0:64, 512:1024])
            ocol = hp * N + it * 512
            nc.vector.tensor_mul(out=out_s[0:64, ocol:ocol + 512],
                                 in0=raw[0:64, 0:512], in1=bcA[0:64, :])
            nc.vector.tensor_mul(out=out_s[64:128, ocol:ocol + 512],
                                 in0=bb[64:128, :], in1=bcB[64:128, :])
            if hp == 1:
                for oc in range(CC):
                    free.append(lambda oc=oc, it=it: emit_y(oc, it))

        prev = None
        for hp in range(HPL):
            for it in range(IT):
                c = pass_begin(hp, it, emit_v_inline=(hp == 0 and it == 0))
                if prev is not None:
                    pass_finish(prev)
                emit_sim(c, 1)
                emit_exp(c, 1)
                pass_main(c, pinned[hp * IT + it])
                prev = c
        pass_finish(prev)
        while free:
            free.popleft()()


# ------------------------- host-side shard / gather -------------------------

def _bf16(a):
    import ml_dtypes
    return np.ascontiguousarray(a.astype(ml_dtypes.bfloat16))


def _shard_inputs(x, context, Wq, Wk, Wv, Wo, bo):
    """Build the per-core DRAM images (all [128, free])."""
    def chunk_rows(a):
        n = a.shape[1]
        return np.ascontiguousarray(
            a.reshape(-1, P, n).transpose(1, 0, 2).reshape(P, -1))

    WqT, WkT, WvT, WoT = Wq.T, Wk.T, Wv.T, Wo.T
    zeros_bo = np.zeros((P, CC), np.float32)

    in_maps = []
    for c in range(N_CORES):
        b, hg = c // 2, c % 2
        cols = slice(hg * CIN, (hg + 1) * CIN)
        x_s = x[b].reshape(CC, P, IT, 512).transpose(1, 2, 0, 3).reshape(P, IT * CC * 512)
        ctx_s = context[b].reshape(CC, P, NT, 512).transpose(1, 2, 0, 3).reshape(P, NT * CC * 512)
        in_maps.append({
            "x": _bf16(x_s),
            "ctx": _bf16(ctx_s),
            "wq": _bf16(chunk_rows(np.ascontiguousarray(WqT[:, cols]))),
            "wk": _bf16(chunk_rows(np.ascontiguousarray(WkT[:, cols]))),
            "wv": _bf16(chunk_rows(np.ascontiguousarray(WvT[:, cols]))),
            "wo": _bf16(chunk_rows(np.ascontiguousarray(WoT[hg * CIN:(hg + 1) * CIN, :]))),
            "bo": np.ascontiguousarray(bo.reshape(CC, P).T) if hg == 0 else zeros_bo,
        })
    return in_maps


def _gather_outputs(results):
    y_full = np.empty((4, C, N), np.float32)
    for b in range(4):
        acc = None
        for hg in range(2):
            y_s = results[2 * b + hg]["y"]                    # [128, 4*2048]
            part = y_s.reshape(P, CC, N).transpose(1, 0, 2).reshape(C, N)
            acc = part if acc is None else acc + part
        y_full[b] = acc
    return y_full


_PROGRAM = None


def _get_program():
    global _PROGRAM
    if _PROGRAM is None:
        _PROGRAM = _build_program()
    return _PROGRAM


def run(trace=False, **inputs):
    nc = _get_program()
    in_maps = _shard_inputs(
        np.asarray(inputs["x"], np.float32),
        np.asarray(inputs["context"], np.float32),
        np.asarray(inputs["Wq"], np.float32),
        np.asarray(inputs["Wk"], np.float32),
        np.asarray(inputs["Wv"], np.float32),
        np.asarray(inputs["Wo"], np.float32),
        np.asarray(inputs["bo"], np.float32),
    )
    res = run_bass_kernel_spmd(nc, in_maps, list(range(N_CORES)), trace=trace)
    return _gather_outputs(res.results), res


def kernel(**inputs):
    out, _ = run(trace=False, **inputs)
    return out


# revision 20
# speedup vs baseline: 1.1705x; 1.0157x over previous
"""Trainium2 Bass kernel for nn_CrossAttention (B=4, H=8, D=64, C=512, N=M=2048).

Sharding: 8 cores = batch (4) x head-group (2). Core c handles batch b=c//2
and heads hg*4..hg*4+4 with hg=c%2 (tensor parallel on inner_dim). Each core
emits a full-shape partial y (its Wo column block times its heads' attention
output, plus bias on hg==0); the host unshard sums the two partials per batch.

V2 design notes (evolved from the fp32r baseline, 211us -> target ~125us):
  * Everything on the PE runs bf16 (fp32 "HIGH" mode streams ~1.33 cyc/col;
    bf16 streams 1.0 and enables FWL weight loads).  Exp output p is bf16, so
    the pv matmuls are bf16 too.
  * pv keeps the ones-augmented lhsT (row 64 = softmax denominator riding the
    matmul for free) and accumulates fp32 in PSUM across the 16 j-chunks.
  * y projection accumulates both inner chunks in PSUM (start/stop pair), no
    SBUF y accumulator.
  * Optionally (SCHRAUD_JS) some exp chunks run on the DVE as a Schraudolph
    bit-trick (i16 = round(A*s + B) viewed as bf16 ~= e^s), offloading the
    scalar engine, which is otherwise the ~1100ns/j bottleneck vs the PE's
    ~840ns/j.

Per-core math:
  q  = Wq[hg] @ x             [256, 2048]
  k  = Wk[hg] @ ctx           [256, 2048]
  vT = ctx.T @ Wv[hg].T       [2048, 256]  (stored j-chunked, ones-augmented)
  per local head h: simT[j,i] = sum_d k[d,j] q[d,i];  p = exp(simT/8)
  out_aug = [vT_h | 1].T @ p  [65, 2048]   (row 64 = softmax denominator)
  out_h   = out_aug[:64] / out_aug[64]
  y_part = Wo[:, hg cols] @ out (+ bo)     [512, 2048]
"""

from collections import deque
from contextlib import ExitStack

import numpy as np

import concourse.bass as bass
import concourse.mybir as mybir
import concourse.tile as tile
from concourse import bacc
from concourse.bass_utils import run_bass_kernel_spmd

FP = mybir.dt.float32
BF16 = mybir.dt.bfloat16
I16 = mybir.dt.int16
EXP = mybir.ActivationFunctionType.Exp

P = 128
H, D = 8, 64
C = 512             # query_dim == full inner_dim
N, M = 2048, 2048
HL = 4              # local heads per core
HPL = 2             # local head pairs
CIN = HL * D        # local inner dim = 256
CC = C // P         # 4 contraction chunks for q/k/v projections
IT = N // 512       # 4 query i-tiles
JC = M // P         # 16 context chunks
GC = JC // 2        # 8 j-chunk pairs per pass (one exp instr each)
NT = M // 512       # 4 context column blocks
ICY = CIN // P      # 2 inner chunks for the y projection
SCALE = float(D) ** -0.5
N_CORES = 8
NWARM = 6

# j-chunk indices (0..15) whose exp runs on the DVE via the Schraudolph
# bit-trick instead of the scalar engine.  () = all-ACT (exact exp).
SCHRAUD_JS = ()
# i16 = SCH_A * s + SCH_B, bitcast to bf16 ~= exp(SCALE * s).
SCH_A = SCALE * (2.0 ** 7) / float(np.log(2.0))
SCH_B = 127.0 * (2.0 ** 7) - 5.72


def _build_program():
    nc = bacc.Bacc("TRN2")
    x = nc.dram_tensor("x", [P, IT * CC * 512], BF16, kind="ExternalInput")
    ctx = nc.dram_tensor("ctx", [P, NT * CC * 512], BF16, kind="ExternalInput")
    wq = nc.dram_tensor("wq", [P, CC * CIN], BF16, kind="ExternalInput")
    wk = nc.dram_tensor("wk", [P, CC * CIN], BF16, kind="ExternalInput")
    wv = nc.dram_tensor("wv", [P, CC * CIN], BF16, kind="ExternalInput")
    wo = nc.dram_tensor("wo", [P, ICY * 512], BF16, kind="ExternalInput")
    bo = nc.dram_tensor("bo", [P, CC], FP, kind="ExternalInput")
    y = nc.dram_tensor("y", [P, CC * N], BF16, kind="ExternalOutput")

    with tile.TileContext(nc) as tc:
        _emit(tc, x, ctx, wq, wk, wv, wo, bo, y)
    nc.finalize()
    return nc


def _emit(tc, x, ctx, wq, wk, wv, wo, bo, y):
    nc = tc.nc
    with ExitStack() as st:
        wpool = st.enter_context(tc.tile_pool(name="weights", bufs=1))
        apool = st.enter_context(tc.tile_pool(name="acts", bufs=1))
        ppool = st.enter_context(tc.tile_pool(name="pexp", bufs=3))
        spool = st.enter_context(tc.tile_pool(name="small", bufs=2))
        ypool = st.enter_context(tc.tile_pool(name="ystage", bufs=4))
        psim = st.enter_context(tc.tile_pool(name="psim", bufs=2, space="PSUM"))
        ppv = st.enter_context(tc.tile_pool(name="ppv", bufs=2, space="PSUM"))
        pmisc = st.enter_context(tc.tile_pool(name="pmisc", bufs=2, space="PSUM"))

        # ---- input loads: ONE queue, strict priority order.  HBM bandwidth
        # is shared by all 8 cores, so splitting across queues only lets the
        # non-critical loads steal bandwidth from the critical ones.  The
        # first sim needs wq+x0 (q proj) then wk+ctx0 (k proj) then wv. ----
        wq_s = wpool.tile([P, CC * CIN], BF16, tag="wq")
        nc.sync.dma_start(out=wq_s, in_=wq[:, :])
        x_s = apool.tile([P, IT * CC * 512], BF16, tag="x")
        nc.sync.dma_start(out=x_s[:, 0:2048], in_=x[:, 0:2048])
        wk_s = wpool.tile([P, CC * CIN], BF16, tag="wk")
        nc.sync.dma_start(out=wk_s, in_=wk[:, :])
        ctx_s = apool.tile([P, NT * CC * 512], BF16, tag="ctx")
        nc.sync.dma_start(out=ctx_s[:, 0:2048], in_=ctx[:, 0:2048])
        wv_s = wpool.tile([P, CC * CIN], BF16, tag="wv")
        nc.sync.dma_start(out=wv_s, in_=wv[:, :])
        for b in range(1, NT):
            nc.sync.dma_start(
                out=ctx_s[:, b * 2048:(b + 1) * 2048],
                in_=ctx[:, b * 2048:(b + 1) * 2048],
            )
            nc.sync.dma_start(
                out=x_s[:, b * 2048:(b + 1) * 2048],
                in_=x[:, b * 2048:(b + 1) * 2048],
            )
        wo_s = wpool.tile([P, ICY * 512], BF16, tag="wo")
        nc.sync.dma_start(out=wo_s, in_=wo[:, :])
        bo_s = wpool.tile([P, CC], FP, tag="bo")
        nc.sync.dma_start(out=bo_s, in_=bo[:, :])

        # ---- persistent SBUF intermediates ----
        # q/k: local head pair hp at cols hp*2048 + it(or nt)*512 + n
        q_s = apool.tile([P, HPL * N], BF16, tag="q")
        k_s = apool.tile([P, HPL * M], BF16, tag="k")
        # v aug: j-chunk j at cols j*(HL*65), local head h at sub-cols h*65
        vaug = apool.tile([P, JC * (HL * 65)], BF16, tag="vaug")
        # attention out, local inner chunk ic at cols ic*2048 + it*512
        out_s = apool.tile([P, ICY * N], BF16, tag="out")
        ones_s = wpool.tile([P, P], BF16, tag="ones")
        nc.vector.memset(ones_s, 1.0)
        ones_f = wpool.tile([P, P], FP, tag="onesf")
        nc.vector.memset(ones_f, 1.0)
        vaug4 = vaug.rearrange("p (j h e) -> p j h e", j=JC, h=HL)
        ones4 = ones_s[:, 0:JC * HL].rearrange("p (j h e) -> p j h e", j=JC, h=HL)
        nc.vector.tensor_copy(out=vaug4[:, :, :, 64:65], in_=ones4)

        # HAM warmup: burn matmuls on the ones tile during the initial DMA
        # wait so the first projections run at the full 2.4GHz clock.
        warm = pmisc.tile([P, 512], FP, tag="scratch", name="warm")
        for w in range(NWARM):
            nc.tensor.matmul(warm[:, 0:P], lhsT=ones_s[:, 0:P],
                             rhs=ones_s[:, 0:P],
                             start=(w == 0), stop=(w == NWARM - 1))
        warm_sink = spool.tile([P, P], FP, tag="warmsink", bufs=1)
        nc.vector.tensor_copy(out=warm_sink, in_=warm[:, 0:P])

        def proj_qk(dst, w_s, oc, rhs_of_cc):
            """One [128, 512] q/k projection tile (local head pair oc)."""
            pt = pmisc.tile([P, 512], FP, tag="scratch")
            for cc in range(CC):
                nc.tensor.matmul(
                    pt,
                    lhsT=w_s[:, cc * CIN + oc * P: cc * CIN + (oc + 1) * P],
                    rhs=rhs_of_cc(cc),
                    start=(cc == 0), stop=(cc == CC - 1),
                )
            nc.vector.tensor_copy(out=dst, in_=pt)

        def emit_q(oc, it):
            proj_qk(q_s[:, oc * N + it * 512: oc * N + (it + 1) * 512], wq_s, oc,
                    lambda cc: x_s[:, it * 2048 + cc * 512: it * 2048 + (cc + 1) * 512])

        def emit_k(oc, nt):
            proj_qk(k_s[:, oc * M + nt * 512: oc * M + (nt + 1) * 512], wk_s, oc,
                    lambda cc: ctx_s[:, nt * 2048 + cc * 512: nt * 2048 + (cc + 1) * 512])

        def emit_v(j):
            nb, jm = j // 4, j % 4
            pt = pmisc.tile([P, 512], FP, tag="scratch")
            for cc in range(CC):
                nc.tensor.matmul(
                    pt[:, 0:CIN],
                    lhsT=ctx_s[:, nb * 2048 + cc * 512 + jm * P:
                               nb * 2048 + cc * 512 + (jm + 1) * P],
                    rhs=wv_s[:, cc * CIN:(cc + 1) * CIN],
                    start=(cc == 0), stop=(cc == CC - 1),
                )
            nc.vector.tensor_copy(
                out=vaug4[:, j, :, 0:64],
                in_=pt[:, 0:CIN].rearrange("p (h e) -> p h e", h=HL),
            )

        def emit_y(oc, nt2):
            """Full y output chunk: both inner chunks accumulated in PSUM."""
            pt = pmisc.tile([P, 512], FP, tag="scratch")
            for ic in range(ICY):
                nc.tensor.matmul(
                    pt,
                    lhsT=wo_s[:, ic * 512 + oc * P: ic * 512 + (oc + 1) * P],
                    rhs=out_s[:, ic * N + nt2 * 512: ic * N + (nt2 + 1) * 512],
                    start=(ic == 0), stop=(ic == ICY - 1),
                )
            ys = ypool.tile([P, 512], BF16, tag="ys")
            nc.vector.tensor_scalar_add(out=ys, in0=pt, scalar1=bo_s[:, oc:oc + 1])
            nc.sync.dma_start(
                out=y[:, oc * N + nt2 * 512: oc * N + (nt2 + 1) * 512], in_=ys)

        # pinned[i]: projection tiles that MUST be emitted during pass i
        # (they feed later js of pass i or pass i+1); free: y chunks drained
        # opportunistically.
        pinned = {i: deque() for i in range(HPL * IT)}
        pinned[0].append(lambda: emit_k(0, 1))
        pinned[0].append(lambda: emit_k(0, 2))
        pinned[0].append(lambda: emit_k(0, 3))
        pinned[0].append(lambda: emit_q(0, 1))
        pinned[1].append(lambda: emit_q(0, 2))
        pinned[1].append(lambda: emit_k(1, 0))
        pinned[1].append(lambda: emit_k(1, 1))
        pinned[2].append(lambda: emit_q(0, 3))
        pinned[2].append(lambda: emit_k(1, 2))
        pinned[2].append(lambda: emit_k(1, 3))
        pinned[2].append(lambda: emit_q(1, 0))
        for it in range(1, IT):
            pinned[2 + it].append(lambda it=it: emit_q(1, it))
        free = deque()

        # upfront: only what pass (0,0)'s first js need
        emit_q(0, 0)
        emit_k(0, 0)

        def pass_begin(hp, it, emit_v_inline):
            """Allocate pass state and emit sim+exp for j=0 (hoisted above the
            previous pass's pv(15)+norm so the ACT exp stream never stalls at
            a pass boundary)."""
            c = {}
            c["hp"], c["it"], c["v_inline"] = hp, it, emit_v_inline
            c["pvA"] = ppv.tile([65, 512], FP, tag="pv", name="pvA")
            c["pvB"] = ppv.tile([65, 512], FP, tag="pv", name="pvB")
            c["qA"] = q_s[0:64, hp * N + it * 512: hp * N + (it + 1) * 512]
            c["qB"] = q_s[64:128, hp * N + it * 512: hp * N + (it + 1) * 512]
            c["jtiles"] = [None] * JC
            emit_sim(c, 0)
            emit_exp(c, 0)
            return c

        def emit_sim(c, j):
            # sim for j-chunk: 2 packed matmuls (heads A/B on row groups
            # 0-1 / 2-3) into one [128, 1024] fp32 psum tile (2 banks).
            hp = c["hp"]
            stt = psim.tile([P, 1024], FP, tag="sim", name="st_t")
            pt = ppool.tile([P, 1024], BF16, tag="p", name="pt")
            c["jtiles"][j] = (stt, pt)
            for half in range(2):
                nc.tensor.matmul(
                    stt[:, half * 512:(half + 1) * 512],
                    lhsT=k_s[half * 64:(half + 1) * 64,
                             hp * M + j * P: hp * M + (j + 1) * P],
                    rhs=(c["qA"] if half == 0 else c["qB"]),
                )
            if c["v_inline"]:
                emit_v(j)

        def emit_exp(c, j):
            stt, pt = c["jtiles"][j]
            if j in SCHRAUD_JS:
                nc.vector.tensor_scalar(
                    out=pt.bitcast(I16), in0=stt,
                    scalar1=SCH_A, scalar2=SCH_B,
                    op0=mybir.AluOpType.mult, op1=mybir.AluOpType.add)
            else:
                nc.scalar.activation(out=pt, in_=stt, func=EXP, scale=SCALE)

        def emit_pv(c, j):
            pt = c["jtiles"][j][1]
            hA, hB = 2 * c["hp"], 2 * c["hp"] + 1
            for half in range(2):
                h = hA if half == 0 else hB
                nc.tensor.matmul(
                    c["pvA"] if half == 0 else c["pvB"],
                    lhsT=vaug[:, j * (HL * 65) + h * 65:
                              j * (HL * 65) + h * 65 + 65],
                    rhs=pt[:, half * 512:(half + 1) * 512],
                    start=(j == 0), stop=(j == JC - 1),
                )

        def pass_main(c, mine):
            for j in range(JC - 1):
                if j > 0:  # (sim(1), exp(1)) follow the previous pass finish
                    emit_sim(c, j + 1)
                    emit_exp(c, j + 1)
                emit_pv(c, j)
                if j % 2 == 1:
                    if mine:
                        mine.popleft()()
                    elif free:
                        free.popleft()()

        def pass_finish(c):
            """pv(15) + pv drains + normalization (emitted after the NEXT
            pass's first sim+exp).  The softmax denominators sit at psum
            partition 64 of pvA/pvB; reciprocal runs in-lane, then a K=1
            matmul with a ones lhsT broadcasts 1/den across all partitions
            (cheaper and lower-latency than gpsimd partition_broadcast)."""
            emit_pv(c, JC - 1)
            pvA, pvB, hp, it = c["pvA"], c["pvB"], c["hp"], c["it"]
            raw = spool.tile([P, 1024], FP, tag="raw", bufs=2)
            nc.vector.tensor_copy(out=raw[0:65, 0:512], in_=pvA)
            nc.vector.tensor_copy(out=raw[0:65, 512:1024], in_=pvB)
            den = spool.tile([1, 1024], FP, tag="den", bufs=2)
            nc.sync.dma_start(out=den, in_=raw[64:65, 0:1024])
            nc.vector.reciprocal_approx_fast(out=den[0:1, :], in_=den[0:1, :])
            bc = spool.tile([P, 1024], FP, tag="bc", bufs=2)
            nc.gpsimd.partition_broadcast(bc, den[0:1, :])
            bb = spool.tile([P, 512], FP, tag="bshift", bufs=2)
            nc.sync.dma_start(out=bb[64:128, :], in_=raw[0:64, 512:1024])
            ocol = hp * N + it * 512
            nc.vector.tensor_mul(out=out_s[0:64, ocol:ocol + 512],
                                 in0=raw[0:64, 0:512], in1=bc[0:64, 0:512])
            nc.vector.tensor_mul(out=out_s[64:128, ocol:ocol + 512],
                                 in0=bb[64:128, :], in1=bc[64:128, 512:1024])
            if hp == 1:
                for oc in range(CC):
                    free.append(lambda oc=oc, it=it: emit_y(oc, it))

        prev = None
        for hp in range(HPL):
            for it in range(IT):
                c = pass_begin(hp, it, emit_v_inline=(hp == 0 and it == 0))
                if prev is not None:
                    pass_finish(prev)
                emit_sim(c, 1)
                emit_exp(c, 1)
                pass_main(c, pinned[hp * IT + it])
                prev = c
        pass_finish(prev)
        while free:
            free.popleft()()


# ------------------------- host-side shard / gather -------------------------

def _bf16(a):
    import ml_dtypes
    return np.ascontiguousarray(a.astype(ml_dtypes.bfloat16))


def _shard_inputs(x, context, Wq, Wk, Wv, Wo, bo):
    """Build the per-core DRAM images (all [128, free])."""
    def chunk_rows(a):
        n = a.shape[1]
        return np.ascontiguousarray(
            a.reshape(-1, P, n).transpose(1, 0, 2).reshape(P, -1))

    WqT, WkT, WvT, WoT = Wq.T, Wk.T, Wv.T, Wo.T
    zeros_bo = np.zeros((P, CC), np.float32)

    in_maps = []
    for c in range(N_CORES):
        b, hg = c // 2, c % 2
        cols = slice(hg * CIN, (hg + 1) * CIN)
        x_s = x[b].reshape(CC, P, IT, 512).transpose(1, 2, 0, 3).reshape(P, IT * CC * 512)
        ctx_s = context[b].reshape(CC, P, NT, 512).transpose(1, 2, 0, 3).reshape(P, NT * CC * 512)
        in_maps.append({
            "x": _bf16(x_s),
            "ctx": _bf16(ctx_s),
            "wq": _bf16(chunk_rows(np.ascontiguousarray(WqT[:, cols]))),
            "wk": _bf16(chunk_rows(np.ascontiguousarray(WkT[:, cols]))),
            "wv": _bf16(chunk_rows(np.ascontiguousarray(WvT[:, cols]))),
            "wo": _bf16(chunk_rows(np.ascontiguousarray(WoT[hg * CIN:(hg + 1) * CIN, :]))),
            "bo": np.ascontiguousarray(bo.reshape(CC, P).T) if hg == 0 else zeros_bo,
        })
    return in_maps


def _gather_outputs(results):
    y_full = np.empty((4, C, N), np.float32)
    for b in range(4):
        acc = None
        for hg in range(2):
            y_s = np.asarray(results[2 * b + hg]["y"], np.float32)  # [128, 4*2048]
            part = y_s.reshape(P, CC, N).transpose(1, 0, 2).reshape(C, N)
            acc = part if acc is None else acc + part
        y_full[b] = acc
    return y_full


_PROGRAM = None


def _get_program():
    global _PROGRAM
    if _PROGRAM is None:
        _PROGRAM = _build_program()
    return _PROGRAM


def run(trace=False, **inputs):
    nc = _get_program()
    in_maps = _shard_inputs(
        np.asarray(inputs["x"], np.float32),
        np.asarray(inputs["context"], np.float32),
        np.asarray(inputs["Wq"], np.float32),
        np.asarray(inputs["Wk"], np.float32),
        np.asarray(inputs["Wv"], np.float32),
        np.asarray(inputs["Wo"], np.float32),
        np.asarray(inputs["bo"], np.float32),
    )
    res = run_bass_kernel_spmd(nc, in_maps, list(range(N_CORES)), trace=trace)
    return _gather_outputs(res.results), res


def kernel(**inputs):
    out, _ = run(trace=False, **inputs)
    return out
